# revision 19
# baseline (speedup 1.0000x reference)
"""Bahdanau additive attention kernel for Trainium2 (8 NeuronCores, SPMD).

Shapes (hardcoded): B=32, TO=1, TI=4096, D=512.
Sharding: data-parallel over batch, 4 batches per core, no collectives.

Per-core algorithm (per batch b; heavy tensors fp16 on chip, fp32 PSUM):
  ctx loaded as [128t, 512d] tiles (cast f32->fp16 during DMA),
  PE-transposed to ctxT [128din, t] so the keys matmul contracts din:
    keysT[dout, t] = WkT.T @ ctxT  (PSUM fp32)
  feats = tanh(keysT + qq[dout]) via ScalarE (qq as per-partition fp32 bias),
  scores[t] = Wv . feats as 4 column-tiled matvec streams (partitions 0/32/64/96),
  e = exp(scores + bv - 4) via one ScalarE op (accum_out -> per-partition sums),
  e transposed to columns; mix = sum_t e_t * ctx[t,:] via 4 column-tiled streams,
  partials summed with a ones-matvec, normalized by 1/sum(e);
  out = tanh([mix, output] @ Wo.T + bo).
"""

import numpy as np

import concourse.bacc as bacc
import concourse.mybir as mybir
import concourse.tile as tile
from concourse.bass_utils import run_bass_kernel_spmd
from concourse.masks import make_identity

dt = mybir.dt
AF = mybir.ActivationFunctionType

B, TO, TI, D = 32, 1, 4096, 512
N_CORES = 8
BPC = B // N_CORES          # batches per core
NG = 2                      # t-groups per batch
TG = TI // NG               # 2048 t per group
NTC = TG // 128             # 16 t128-chunks per group
NTS = TG // 512             # 4 t512-chunks per group
DC = D // 128               # 4 d-chunks
KO = 2 * D // 128           # 8 k-chunks for the output projection
ESHIFT = -4.0               # exp(scores + bv + ESHIFT): keeps e in fp16 range

_CACHE = {}


def _build():
    nc = bacc.Bacc("TRN2", target_bir_lowering=False, debug=False)

    ctx_d = nc.dram_tensor("context", [BPC, TI, D], dt.float32, kind="ExternalInput")
    outp_d = nc.dram_tensor("output", [BPC, TO, D], dt.float32, kind="ExternalInput")
    wq_d = nc.dram_tensor("Wq", [D, D], dt.float32, kind="ExternalInput")
    bq_d = nc.dram_tensor("bq", [D], dt.float32, kind="ExternalInput")
    wk_d = nc.dram_tensor("Wk", [D, D], dt.float32, kind="ExternalInput")
    bk_d = nc.dram_tensor("bk", [D], dt.float32, kind="ExternalInput")
    wv_d = nc.dram_tensor("Wv", [1, D], dt.float32, kind="ExternalInput")
    bv_d = nc.dram_tensor("bv", [1], dt.float32, kind="ExternalInput")
    wo_d = nc.dram_tensor("Wo", [D, 2 * D], dt.float32, kind="ExternalInput")
    bo_d = nc.dram_tensor("bo", [D], dt.float32, kind="ExternalInput")

    o_out = nc.dram_tensor("o_out", [BPC, TO, D], dt.float32, kind="ExternalOutput")
    o_attn = nc.dram_tensor("o_attn", [BPC, TO, TI], dt.float32, kind="ExternalOutput")
    o_mix = nc.dram_tensor("o_mix", [BPC, TO, D], dt.float32, kind="ExternalOutput")

    with tile.TileContext(nc) as tc:
        _emit(nc, tc, ctx_d, outp_d, wq_d, bq_d, wk_d, bk_d, wv_d, bv_d,
              wo_d, bo_d, o_out, o_attn, o_mix)
    nc.compile()
    return nc


def _emit(nc, tc, ctx_d, outp_d, wq_d, bq_d, wk_d, bk_d, wv_d, bv_d,
          wo_d, bo_d, o_out, o_attn, o_mix):
    f32, f16 = dt.float32, dt.float16

    with tc.tile_pool(name="persist", bufs=1) as pp:
        # ---- ctx pool + earliest possible prefetch of the first group ----
        ctx_pool_cm = tc.tile_pool(name="ctx", bufs=4)
        ctx_pool = ctx_pool_cm.__enter__()

        def load_ctx(b, g, nsplit=4):
            ctx_g = ctx_pool.tile([128, NTC, D], f16, tag="ctx", name="ctx_g")
            step = NTC // nsplit
            for q in range(nsplit):
                t0 = g * TG + q * step * 128
                nc.gpsimd.dma_start(
                    out=ctx_g[:, q * step:(q + 1) * step, :],
                    in_=ctx_d[b, t0:t0 + step * 128, :].rearrange(
                        "(n p) d -> p n d", p=128))
            return ctx_g

        groups = [(b, g) for b in range(BPC) for g in range(NG)]
        prefetched = {groups[0]: load_ctx(*groups[0], nsplit=8)}

        # ---- constants ----
        ident_f = pp.tile([128, 128], f32, tag="ident_f")
        make_identity(nc, ident_f[:])
        ident_h = pp.tile([128, 128], f16, tag="ident_h")
        nc.vector.tensor_copy(ident_h[:], ident_f[:])

        ones_f = pp.tile([128, 128], f32, tag="ones_f")
        nc.gpsimd.memset(ones_f[:], 1.0)
        ones_col_h = pp.tile([128, 1], f16, tag="ones_col_h")
        nc.vector.tensor_copy(ones_col_h[:], ones_f[:, 0:1])
        one1_h = pp.tile([1, 1], f16, tag="one1_h")
        nc.vector.tensor_copy(one1_h[:], ones_f[0:1, 0:1])

        mask_f = pp.tile([128, 1], f32, tag="mask_f")   # 1 at rows 0/32/64/96
        nc.gpsimd.memset(mask_f[:], 0.0)
        for j in range(4):
            nc.gpsimd.memset(mask_f[32 * j:32 * j + 1, 0:1], 1.0)
        maskv = pp.tile([128, 1], f16, tag="maskv")
        nc.vector.tensor_copy(maskv[:], mask_f[:])

        # ---- small weight loads ----
        bq_sb = pp.tile([1, D], f32, tag="bq")
        bk_sb = pp.tile([1, D], f32, tag="bk")
        bo_sb = pp.tile([1, D], f32, tag="bo")
        bv_sb = pp.tile([1, 1], f32, tag="bv")
        wv_sb = pp.tile([1, D], f32, tag="wv")
        outp_sb = pp.tile([BPC, D], f32, tag="outp")
        nc.sync.dma_start(out=bq_sb[0:1, :], in_=bq_d[:])
        nc.sync.dma_start(out=bk_sb[0:1, :], in_=bk_d[:])
        nc.sync.dma_start(out=bo_sb[0:1, :], in_=bo_d[:])
        nc.sync.dma_start(out=bv_sb[0:1, :], in_=bv_d[:])
        nc.sync.dma_start(out=wv_sb[:], in_=wv_d[:])
        nc.sync.dma_start(out=outp_sb[:], in_=outp_d.rearrange("b o d -> (b o) d"))
        outp_row = pp.tile([1, BPC * D], f32, tag="outp_row")
        nc.sync.dma_start(out=outp_row[0:1, :],
                          in_=outp_d.rearrange("b o d -> (b o d)"))

        bqk = pp.tile([1, D], f32, tag="bqk")
        nc.vector.tensor_add(bqk[:], bq_sb[:], bk_sb[:])
        eshift_t = pp.tile([1, 1], f32, tag="eshift_t")
        nc.gpsimd.memset(eshift_t[:], ESHIFT)
        bvs = pp.tile([1, 1], f32, tag="bvs")   # bv + ESHIFT
        nc.vector.tensor_add(bvs[:], bv_sb[:], eshift_t[:])
        bo_h = pp.tile([1, D], f16, tag="bo_h")
        nc.vector.tensor_copy(bo_h[:], bo_sb[:])

        # ---- persistent transformed weights ----
        wkT = pp.tile([128, DC, D], f16, tag="wkT")      # [din_chunk, ., dout]
        woT = pp.tile([128, KO, D], f16, tag="woT")      # [k_chunk, ., dout]
        wvT = pp.tile([128, DC], f16, tag="wvT")         # Wv transposed columns
        qqT = pp.tile([128, DC, BPC], f32, tag="qqT")    # per-batch bias columns
        bvs_bc = pp.tile([128, 1], f32, tag="bvs_bc")    # bv+ESHIFT broadcast

        # ---- precompute scratch (freed before the main loop) ----
        with tc.tile_pool(name="scratch", bufs=1) as sp, \
             tc.tile_pool(name="ps_pre", bufs=2, space="PSUM") as ps_pre:

            # bvs broadcast to [128,1]
            pbv = ps_pre.tile([128, 1], f32, tag="pre_s")
            nc.tensor.matmul(pbv[:], ones_f[0:1, :], bvs[0:1, 0:1],
                             start=True, stop=True)
            nc.vector.tensor_copy(bvs_bc[:], pbv[:])

            # WkT (fp16)
            wk_sb = sp.tile([128, DC, D], f32, tag="wk_sb")
            nc.sync.dma_start(out=wk_sb[:],
                              in_=wk_d.rearrange("(c p) d -> p c d", p=128))
            wk_h = sp.tile([128, DC, D], f16, tag="wk_h")
            nc.vector.tensor_copy(wk_h[:].rearrange("p c d -> p (c d)"),
                                  wk_sb[:].rearrange("p c d -> p (c d)"))
            for i in range(DC):
                pwk = ps_pre.tile([128, D], f16, tag="pre_h")
                for c in range(DC):
                    nc.tensor.transpose(pwk[:, c * 128:(c + 1) * 128],
                                        wk_h[:, c, i * 128:(i + 1) * 128], ident_h[:])
                nc.vector.tensor_copy(wkT[:, i, :], pwk[:])

            # WoT (fp16)
            wo_sb = sp.tile([128, DC, 2 * D], f32, tag="wo_sb")
            nc.sync.dma_start(out=wo_sb[:],
                              in_=wo_d.rearrange("(c p) k -> p c k", p=128))
            wo_h = sp.tile([128, DC, 2 * D], f16, tag="wo_h")
            nc.vector.tensor_copy(wo_h[:].rearrange("p c k -> p (c k)"),
                                  wo_sb[:].rearrange("p c k -> p (c k)"))
            for i in range(KO):
                pwo = ps_pre.tile([128, D], f16, tag="pre_h")
                for c in range(DC):
                    nc.tensor.transpose(pwo[:, c * 128:(c + 1) * 128],
                                        wo_h[:, c, i * 128:(i + 1) * 128], ident_h[:])
                nc.vector.tensor_copy(woT[:, i, :], pwo[:])

            # WvT (fp16)
            pwv = ps_pre.tile([128, DC], f32, tag="pre_s")
            for c in range(DC):
                nc.tensor.transpose(pwv[:, c:c + 1], wv_sb[0:1, c * 128:(c + 1) * 128],
                                    ident_f[0:1, 0:1])
            nc.vector.tensor_copy(wvT[:], pwv[:])

            # qq = output @ Wq.T + bq + bk  (fp32, one-off), transposed columns
            wq_sb = sp.tile([128, DC, D], f32, tag="wq_sb")
            nc.sync.dma_start(out=wq_sb[:],
                              in_=wq_d.rearrange("(c p) d -> p c d", p=128))
            wqT = sp.tile([128, DC, D], f32, tag="wqT")
            for i in range(DC):
                pwq = ps_pre.tile([128, D], f32, tag="pre_f")
                for c in range(DC):
                    nc.tensor.transpose(pwq[:, c * 128:(c + 1) * 128],
                                        wq_sb[:, c, i * 128:(i + 1) * 128], ident_f[:])
                nc.vector.tensor_copy(wqT[:, i, :], pwq[:])
            outT = sp.tile([128, DC, BPC], f32, tag="outT")
            pot = ps_pre.tile([128, DC * BPC], f32, tag="pre_s")
            for i in range(DC):
                nc.tensor.transpose(pot[:, i * BPC:(i + 1) * BPC],
                                    outp_sb[0:BPC, i * 128:(i + 1) * 128],
                                    ident_f[0:BPC, 0:BPC])
            nc.vector.tensor_copy(outT[:].rearrange("p c b -> p (c b)"), pot[:])
            ones14 = sp.tile([1, BPC], f32, tag="ones14")
            nc.gpsimd.memset(ones14[:], 1.0)
            pqq = ps_pre.tile([BPC, D], f32, tag="pre_q")
            for i in range(DC):
                nc.tensor.matmul(pqq[:], outT[:, i, :], wqT[:, i, :],
                                 start=(i == 0), stop=False)
            nc.tensor.matmul(pqq[:], ones14[:], bqk[:], start=False, stop=True)
            qq_sb = sp.tile([BPC, D], f32, tag="qq_sb")
            nc.vector.tensor_copy(qq_sb[:], pqq[:])
            pqt = ps_pre.tile([128, DC * BPC], f32, tag="pre_s")
            for c in range(DC):
                nc.tensor.transpose(pqt[:, c * BPC:(c + 1) * BPC],
                                    qq_sb[0:BPC, c * 128:(c + 1) * 128],
                                    ident_f[0:BPC, 0:BPC])
            nc.vector.tensor_copy(qqT[:].rearrange("p c b -> p (c b)"), pqt[:])

        # ---- main loop (software-pipelined: group chain deferred one step) ----
        with tc.tile_pool(name="ctxT", bufs=2) as ctxT_pool, \
             tc.tile_pool(name="feats", bufs=2) as feats_pool, \
             tc.tile_pool(name="sb_small", bufs=2) as sbs, \
             tc.tile_pool(name="ps_k", bufs=2, space="PSUM") as ps_k, \
             tc.tile_pool(name="ps_tr", bufs=2, space="PSUM") as ps_tr, \
             tc.tile_pool(name="ps_m", bufs=1, space="PSUM") as ps_m, \
             tc.tile_pool(name="ps_s", bufs=1, space="PSUM") as ps_s:

            bstate = {}

            def start_batch(b):
                psum_m = ps_m.tile([128, D], f32, tag="mix", name="psum_m")
                nc.scalar.memzero(psum_m[:])
                bstate[b] = {
                    "psum_m": psum_m,
                    "attn_cols": sbs.tile([128, NG * NTC], f32,
                                          tag="attn_cols", name="attn_cols"),
                    "esums": sbs.tile([128, NG], f32, tag="esums", name="esums"),
                }

            def emit_tr_quad(ctx_g, ctxT_g, q, dc):
                ptr = ps_tr.tile([128, 512], f16, tag="tr", name="ptr")
                for j in range(4):
                    n = q * 4 + j
                    nc.tensor.transpose(
                        ptr[:, j * 128:(j + 1) * 128],
                        ctx_g[:, n, dc * 128:(dc + 1) * 128],
                        ident_h[:])
                nc.vector.tensor_copy(
                    ctxT_g[:, dc, q * 512:(q + 1) * 512], ptr[:])

            def emit_keys_unit(b, ctxT_g, feats_g, mc, h):
                pk = ps_k.tile([128, TG // 2], f32, tag="k", name="pk")
                for kc in range(DC):
                    for th in range(2):
                        t0 = th * 512
                        nc.tensor.matmul(
                            pk[:, t0:t0 + 512],
                            wkT[:, kc, mc * 128:(mc + 1) * 128],
                            ctxT_g[:, kc, h * 1024 + t0:h * 1024 + t0 + 512],
                            start=(kc == 0), stop=(kc == DC - 1),
                            skip_group_check=True)
                nc.scalar.activation(
                    feats_g[:, mc, h * 1024:(h + 1) * 1024], pk[:],
                    AF.Tanh, bias=qqT[:, mc, b:b + 1])

            def chain_scores(b, g, feats_g):
                st = bstate[b]
                psc = ps_s.tile([128, 512], f32, tag="sc", name="psc")
                nc.scalar.memzero(psc[:])
                for mc in range(DC):
                    for j in range(NTS):
                        nc.tensor.matmul(
                            psc[32 * j:32 * j + 1, :],
                            wvT[:, mc:mc + 1],
                            feats_g[:, mc, j * 512:(j + 1) * 512],
                            start=(mc == 0), stop=(mc == DC - 1),
                            tile_position=(0, 32 * j),
                            skip_group_check=True)
                e_sb = sbs.tile([128, 512], f32, tag="e_sb", name="e_sb")
                nc.scalar.activation(
                    e_sb[:], psc[:], AF.Exp, bias=bvs_bc[:, 0:1],
                    accum_out=st["esums"][:, g:g + 1])
                return e_sb

            def chain_etr(b, g, e_sb):
                st = bstate[b]
                pe = ps_s.tile([128, NTC], f32, tag="sc", name="pe")
                for j in range(NTS):
                    for k in range(4):
                        n = j * 4 + k
                        nc.tensor.transpose(
                            pe[:, n:n + 1],
                            e_sb[32 * j:32 * j + 1, k * 128:(k + 1) * 128],
                            ident_f[32 * j:32 * j + 1, 32 * j:32 * j + 1],
                            tile_position=(32 * j, 0))
                ecol = sbs.tile([128, NTC], f16, tag="ecol", name="ecol")
                nc.vector.tensor_copy(ecol[:], pe[:])
                nc.vector.tensor_copy(st["attn_cols"][:, g * NTC:(g + 1) * NTC], pe[:])
                return ecol

            def chain_mix(b, g, ctx_g, ecol):
                st = bstate[b]
                for n in range(NTC):
                    j = n % 4
                    nc.tensor.matmul(
                        st["psum_m"][32 * j:32 * j + 1, :],
                        ecol[:, n:n + 1],
                        ctx_g[:, n, :],
                        start=False, stop=(g == NG - 1 and n >= NTC - 4),
                        tile_position=(0, 32 * j),
                        skip_group_check=True)

            def chain(b, g, ctx_g, feats_g):
                e_sb = chain_scores(b, g, feats_g)
                ecol = chain_etr(b, g, e_sb)
                chain_mix(b, g, ctx_g, ecol)

            def tail(b):
                st = bstate.pop(b)
                psum_m, attn_cols, esums = (st["psum_m"], st["attn_cols"],
                                            st["esums"])
                # total sum of e: masked partition-sum of per-partition sums
                esums_h = sbs.tile([128, NG], f16, tag="esums_h")
                nc.vector.tensor_copy(esums_h[:], esums[:])
                pes = ps_s.tile([1, NG], f32, tag="sc", name="pes")
                nc.tensor.matmul(pes[:], maskv[:], esums_h[:],
                                 start=True, stop=True)
                erow = sbs.tile([1, NG], f32, tag="erow")
                nc.vector.tensor_copy(erow[:], pes[:])
                esumT = sbs.tile([1, 1], f32, tag="esumT")
                nc.vector.reduce_sum(esumT[:], erow[:], axis=mybir.AxisListType.X)
                recip = sbs.tile([1, 1], f32, tag="recip")
                nc.vector.reciprocal(recip[:], esumT[:])

                # broadcast recip to [128,1] via tiny fp32 matmul
                pbr = ps_s.tile([128, 1], f32, tag="sc", name="pbr")
                nc.tensor.matmul(pbr[:], ones_f[0:1, :], recip[0:1, 0:1],
                                 start=True, stop=True)
                recip_bc = sbs.tile([128, 1], f32, tag="recip_bc")
                nc.vector.tensor_copy(recip_bc[:], pbr[:])

                # attn = e * recip, transposed back to rows, DMA out
                attn_n = sbs.tile([128, NG * NTC], f32, tag="attn_n")
                nc.scalar.activation(attn_n[:], attn_cols[:], AF.Copy,
                                     scale=recip_bc[:, 0:1])
                pa = ps_s.tile([NG * NTC, 128], f32, tag="sc", name="pa")
                nc.tensor.transpose(pa[:], attn_n[:], ident_f[:])
                attn_out = sbs.tile([NG * NTC, 128], f32, tag="attn_out")
                nc.vector.tensor_copy(attn_out[:], pa[:])
                nc.sync.dma_start(
                    out=o_attn[b].rearrange("o (a c) -> (o a) c", c=128),
                    in_=attn_out[:])

                # mix: sum 4 column partials (garbage rows are zero)
                pmh = sbs.tile([128, D], f16, tag="pmh")
                nc.vector.tensor_copy(pmh[:], psum_m[:])
                pms = ps_s.tile([1, D], f32, tag="sc", name="pms")
                nc.tensor.matmul(pms[:], ones_col_h[:], pmh[:],
                                 start=True, stop=True)
                mix_sb = sbs.tile([1, D], f32, tag="mix_sb")
                nc.scalar.activation(mix_sb[:], pms[0:1, :], AF.Copy,
                                     scale=recip[0:1, 0:1])
                nc.sync.dma_start(out=o_mix[b].rearrange("o d -> (o d)"),
                                  in_=mix_sb[0:1, :])

                # out = tanh([mix, output] @ Wo.T + bo)
                pc = ps_s.tile([128, KO], f32, tag="sc", name="pc")
                for i in range(DC):
                    nc.tensor.transpose(pc[:, i:i + 1],
                                        mix_sb[0:1, i * 128:(i + 1) * 128],
                                        ident_f[0:1, 0:1])
                for i in range(DC):
                    nc.tensor.transpose(
                        pc[:, DC + i:DC + i + 1],
                        outp_row[0:1, b * D + i * 128:b * D + (i + 1) * 128],
                        ident_f[0:1, 0:1])
                comb = sbs.tile([128, KO], f16, tag="comb")
                nc.vector.tensor_copy(comb[:], pc[:])
                po = ps_s.tile([1, D], f32, tag="sc", name="po")
                for i in range(KO):
                    nc.tensor.matmul(po[:], comb[:, i:i + 1], woT[:, i, :],
                                     start=(i == 0), stop=False,
                                     skip_group_check=True)
                nc.tensor.matmul(po[:], one1_h[:], bo_h[:], start=False, stop=True,
                                 skip_group_check=True)
                out_sb = sbs.tile([1, D], f32, tag="out_sb")
                nc.scalar.activation(out_sb[:], po[0:1, :], AF.Tanh)
                nc.sync.dma_start(out=o_out[b].rearrange("o d -> (o d)"),
                                  in_=out_sb[0:1, :])

            NQ = NTC // 4  # tr quad columns per g
            state = {}     # i -> dict(b, g, ctx, ctxT, feats)
            n_groups = len(groups)

            def emit_iteration(i):
                # stage A for group i: transposes; stage B for group i-1: keys;
                # stage C for group i-2: scores/softmax/mix chain (+ batch tail)
                if i < n_groups:
                    b, g = groups[i]
                    if g == 0:
                        start_batch(b)
                    ctx_g = prefetched.pop((b, g))
                    ctxT_g = ctxT_pool.tile([128, DC, TG], f16, tag="ctxT",
                                            name="ctxT_g")
                    state[i] = {"b": b, "g": g, "ctx": ctx_g, "ctxT": ctxT_g}
                    tr_units = [(q, dc) for q in range(NQ) for dc in range(DC)]
                else:
                    tr_units = []
                if 0 <= i - 1 < n_groups:
                    st1 = state[i - 1]
                    st1["feats"] = feats_pool.tile([128, DC, TG], f16,
                                                   tag="feats", name="feats_g")
                    k_units = [(mc, h) for mc in range(DC) for h in range(2)]
                else:
                    st1, k_units = None, []
                # interleave: 2 tr quads per keys unit
                nk = len(k_units)
                for s in range(max(nk, (len(tr_units) + 1) // 2)):
                    for t in tr_units[2 * s:2 * s + 2]:
                        emit_tr_quad(state[i]["ctx"], state[i]["ctxT"], *t)
                    if s < nk:
                        mc, h = k_units[s]
                        emit_keys_unit(st1["b"], st1["ctxT"], st1["feats"], mc, h)
                if i + 1 < n_groups and groups[i + 1] not in prefetched:
                    prefetched[groups[i + 1]] = load_ctx(*groups[i + 1])
                if 0 <= i - 2 < n_groups:
                    st2 = state.pop(i - 2)
                    chain(st2["b"], st2["g"], st2["ctx"], st2["feats"])
                    if st2["g"] == NG - 1:
                        tail(st2["b"])

            for i in range(n_groups):
                emit_iteration(i)
            # drain: keys for the last group, then the final two chains
            # interleaved so their ACT/DVE round-trips hide behind PE work
            stA = state.pop(n_groups - 2)
            stB = state.pop(n_groups - 1)
            stB["feats"] = feats_pool.tile([128, DC, TG], f16,
                                           tag="feats", name="feats_g")
            for mc in range(DC):
                for h in range(2):
                    emit_keys_unit(stB["b"], stB["ctxT"], stB["feats"], mc, h)
            eA = chain_scores(stA["b"], stA["g"], stA["feats"])
            eB = chain_scores(stB["b"], stB["g"], stB["feats"])
            ecA = chain_etr(stA["b"], stA["g"], eA)
            ecB = chain_etr(stB["b"], stB["g"], eB)
            chain_mix(stA["b"], stA["g"], stA["ctx"], ecA)
            chain_mix(stB["b"], stB["g"], stB["ctx"], ecB)
            if stA["g"] == NG - 1:
                tail(stA["b"])
            tail(stB["b"])
        ctx_pool_cm.__exit__(None, None, None)


def _get_nc():
    if "nc" not in _CACHE:
        _CACHE["nc"] = _build()
    return _CACHE["nc"]


def _run_spmd(in_maps, **kw):
    nc = _get_nc()
    return run_bass_kernel_spmd(nc, in_maps, list(range(N_CORES)), **kw)


def _make_in_maps(output, context, Wq, bq, Wk, bk, Wv, bv, Wo, bo):
    arrs = {
        "Wq": np.ascontiguousarray(np.asarray(Wq, np.float32)),
        "bq": np.ascontiguousarray(np.asarray(bq, np.float32)),
        "Wk": np.ascontiguousarray(np.asarray(Wk, np.float32)),
        "bk": np.ascontiguousarray(np.asarray(bk, np.float32)),
        "Wv": np.ascontiguousarray(np.asarray(Wv, np.float32)),
        "bv": np.ascontiguousarray(np.asarray(bv, np.float32)),
        "Wo": np.ascontiguousarray(np.asarray(Wo, np.float32)),
        "bo": np.ascontiguousarray(np.asarray(bo, np.float32)),
    }
    output = np.ascontiguousarray(np.asarray(output, np.float32))
    context = np.ascontiguousarray(np.asarray(context, np.float32))
    in_maps = []
    for c in range(N_CORES):
        sl = slice(c * BPC, (c + 1) * BPC)
        m = dict(arrs)
        m["output"] = np.ascontiguousarray(output[sl])
        m["context"] = np.ascontiguousarray(context[sl])
        in_maps.append(m)
    return in_maps


def kernel(output, context, Wq, bq, Wk, bk, Wv, bv, Wo, bo):
    in_maps = _make_in_maps(output, context, Wq, bq, Wk, bk, Wv, bv, Wo, bo)
    res = _run_spmd(in_maps)
    out = np.concatenate([r["o_out"] for r in res.results], axis=0)
    attn = np.concatenate([r["o_attn"] for r in res.results], axis=0)
    mix = np.concatenate([r["o_mix"] for r in res.results], axis=0)
    return (out, attn, mix)


# revision 20
# speedup vs baseline: 1.1947x; 1.1947x over previous
"""Bahdanau additive attention kernel for Trainium2 (8 NeuronCores, SPMD).

Shapes (hardcoded): B=32, TO=1, TI=4096, D=512.
Sharding: data-parallel over batch, 4 batches per core, no collectives.

Per-core algorithm (per batch b; heavy tensors fp16 on chip, fp32 PSUM):
  ctx loaded as [128t, 512d] tiles (cast f32->fp16 during DMA),
  PE-transposed to ctxT [128din, t] so the keys matmul contracts din:
    keysT[dout, t] = WkT.T @ ctxT  (PSUM fp32)
  feats = tanh(keysT + qq[dout]) via ScalarE (qq as per-partition fp32 bias),
  scores[t] = Wv . feats as 4 column-tiled matvec streams (partitions 0/32/64/96),
  e = exp(scores + bv - 4) via one ScalarE op (accum_out -> per-partition sums),
  e transposed to columns; mix = sum_t e_t * ctx[t,:] via 4 column-tiled streams,
  partials summed with a ones-matvec, normalized by 1/sum(e);
  out = tanh([mix, output] @ Wo.T + bo).
"""

import numpy as np

import concourse.bacc as bacc
import concourse.mybir as mybir
import concourse.tile as tile
from concourse.bass_utils import run_bass_kernel_spmd
from concourse.masks import make_identity

dt = mybir.dt
AF = mybir.ActivationFunctionType

B, TO, TI, D = 32, 1, 4096, 512
N_CORES = 8
BPC = B // N_CORES          # batches per core
NG = 2                      # t-groups per batch
TG = TI // NG               # 2048 t per group
NTC = TG // 128             # 16 t128-chunks per group
NTS = TG // 512             # 4 t512-chunks per group
DC = D // 128               # 4 d-chunks
KO = 2 * D // 128           # 8 k-chunks for the output projection
ESHIFT = -4.0               # exp(scores + bv + ESHIFT): keeps e in fp16 range

_CACHE = {}


def _build():
    nc = bacc.Bacc("TRN2", target_bir_lowering=False, debug=False)

    ctx_d = nc.dram_tensor("context", [BPC, TI, D], dt.float32, kind="ExternalInput")
    outp_d = nc.dram_tensor("output", [BPC, TO, D], dt.float32, kind="ExternalInput")
    wq_d = nc.dram_tensor("Wq", [D, D], dt.float32, kind="ExternalInput")
    bq_d = nc.dram_tensor("bq", [D], dt.float32, kind="ExternalInput")
    wk_d = nc.dram_tensor("Wk", [D, D], dt.float32, kind="ExternalInput")
    bk_d = nc.dram_tensor("bk", [D], dt.float32, kind="ExternalInput")
    wv_d = nc.dram_tensor("Wv", [1, D], dt.float32, kind="ExternalInput")
    bv_d = nc.dram_tensor("bv", [1], dt.float32, kind="ExternalInput")
    wo_d = nc.dram_tensor("Wo", [D, 2 * D], dt.float32, kind="ExternalInput")
    bo_d = nc.dram_tensor("bo", [D], dt.float32, kind="ExternalInput")

    o_out = nc.dram_tensor("o_out", [BPC, TO, D], dt.float32, kind="ExternalOutput")
    o_attn = nc.dram_tensor("o_attn", [BPC, TO, TI], dt.float32, kind="ExternalOutput")
    o_mix = nc.dram_tensor("o_mix", [BPC, TO, D], dt.float32, kind="ExternalOutput")

    with tile.TileContext(nc) as tc:
        _emit(nc, tc, ctx_d, outp_d, wq_d, bq_d, wk_d, bk_d, wv_d, bv_d,
              wo_d, bo_d, o_out, o_attn, o_mix)
    nc.compile()
    return nc


def _emit(nc, tc, ctx_d, outp_d, wq_d, bq_d, wk_d, bk_d, wv_d, bv_d,
          wo_d, bo_d, o_out, o_attn, o_mix):
    f32, f16 = dt.float32, dt.float16

    with tc.tile_pool(name="persist", bufs=1) as pp:
        # ---- ctx pool + earliest possible prefetch of the first group ----
        ctx_pool_cm = tc.tile_pool(name="ctx", bufs=4)
        ctx_pool = ctx_pool_cm.__enter__()

        def load_ctx(b, g, nsplit=4):
            ctx_g = ctx_pool.tile([128, NTC, D], f16, tag="ctx", name="ctx_g")
            step = NTC // nsplit
            for q in range(nsplit):
                t0 = g * TG + q * step * 128
                nc.gpsimd.dma_start(
                    out=ctx_g[:, q * step:(q + 1) * step, :],
                    in_=ctx_d[b, t0:t0 + step * 128, :].rearrange(
                        "(n p) d -> p n d", p=128))
            return ctx_g

        groups = [(b, g) for b in range(BPC) for g in range(NG)]
        prefetched = {groups[0]: load_ctx(*groups[0], nsplit=8)}

        # ---- constants ----
        ident_f = pp.tile([128, 128], f32, tag="ident_f")
        make_identity(nc, ident_f[:])
        ident_h = pp.tile([128, 128], f16, tag="ident_h")
        nc.vector.tensor_copy(ident_h[:], ident_f[:])

        ones_f = pp.tile([128, 128], f32, tag="ones_f")
        nc.gpsimd.memset(ones_f[:], 1.0)
        ones_col_h = pp.tile([128, 1], f16, tag="ones_col_h")
        nc.vector.tensor_copy(ones_col_h[:], ones_f[:, 0:1])
        one1_h = pp.tile([1, 1], f16, tag="one1_h")
        nc.vector.tensor_copy(one1_h[:], ones_f[0:1, 0:1])

        mask_f = pp.tile([128, 1], f32, tag="mask_f")   # 1 at rows 0/32/64/96
        nc.gpsimd.memset(mask_f[:], 0.0)
        for j in range(4):
            nc.gpsimd.memset(mask_f[32 * j:32 * j + 1, 0:1], 1.0)
        maskv = pp.tile([128, 1], f16, tag="maskv")
        nc.vector.tensor_copy(maskv[:], mask_f[:])

        # ---- small weight loads ----
        bq_sb = pp.tile([1, D], f32, tag="bq")
        bk_sb = pp.tile([1, D], f32, tag="bk")
        bo_sb = pp.tile([1, D], f32, tag="bo")
        bv_sb = pp.tile([1, 1], f32, tag="bv")
        wv_sb = pp.tile([1, D], f32, tag="wv")
        outp_sb = pp.tile([BPC, D], f32, tag="outp")
        nc.sync.dma_start(out=bq_sb[0:1, :], in_=bq_d[:])
        nc.sync.dma_start(out=bk_sb[0:1, :], in_=bk_d[:])
        nc.sync.dma_start(out=bo_sb[0:1, :], in_=bo_d[:])
        nc.sync.dma_start(out=bv_sb[0:1, :], in_=bv_d[:])
        nc.sync.dma_start(out=wv_sb[:], in_=wv_d[:])
        nc.sync.dma_start(out=outp_sb[:], in_=outp_d.rearrange("b o d -> (b o) d"))
        outp_row = pp.tile([1, BPC * D], f32, tag="outp_row")
        nc.sync.dma_start(out=outp_row[0:1, :],
                          in_=outp_d.rearrange("b o d -> (b o d)"))

        bqk = pp.tile([1, D], f32, tag="bqk")
        nc.vector.tensor_add(bqk[:], bq_sb[:], bk_sb[:])
        eshift_t = pp.tile([1, 1], f32, tag="eshift_t")
        nc.gpsimd.memset(eshift_t[:], ESHIFT)
        bvs = pp.tile([1, 1], f32, tag="bvs")   # bv + ESHIFT
        nc.vector.tensor_add(bvs[:], bv_sb[:], eshift_t[:])
        bo_h = pp.tile([1, D], f16, tag="bo_h")
        nc.vector.tensor_copy(bo_h[:], bo_sb[:])

        # ---- persistent transformed weights ----
        wkT = pp.tile([128, DC, D], f16, tag="wkT")      # [din_chunk, ., dout]
        woT = pp.tile([128, KO, D], f16, tag="woT")      # [k_chunk, ., dout]
        wvT = pp.tile([128, DC], f16, tag="wvT")         # Wv transposed columns
        qqT = pp.tile([128, DC, BPC], f32, tag="qqT")    # per-batch bias columns
        bvs_bc = pp.tile([128, 1], f32, tag="bvs_bc")    # bv+ESHIFT broadcast

        # ---- precompute scratch (freed before the main loop) ----
        with tc.tile_pool(name="scratch", bufs=1) as sp, \
             tc.tile_pool(name="ps_pre", bufs=2, space="PSUM") as ps_pre:

            # bvs broadcast to [128,1]
            pbv = ps_pre.tile([128, 1], f32, tag="pre_s")
            nc.tensor.matmul(pbv[:], ones_f[0:1, :], bvs[0:1, 0:1],
                             start=True, stop=True)
            nc.vector.tensor_copy(bvs_bc[:], pbv[:])

            # WkT (fp16)
            wk_sb = sp.tile([128, DC, D], f32, tag="wk_sb")
            nc.sync.dma_start(out=wk_sb[:],
                              in_=wk_d.rearrange("(c p) d -> p c d", p=128))
            wk_h = sp.tile([128, DC, D], f16, tag="wk_h")
            nc.vector.tensor_copy(wk_h[:].rearrange("p c d -> p (c d)"),
                                  wk_sb[:].rearrange("p c d -> p (c d)"))
            for i in range(DC):
                pwk = ps_pre.tile([128, D], f16, tag="pre_h")
                for c in range(DC):
                    nc.tensor.transpose(pwk[:, c * 128:(c + 1) * 128],
                                        wk_h[:, c, i * 128:(i + 1) * 128], ident_h[:])
                nc.vector.tensor_copy(wkT[:, i, :], pwk[:])

            # WoT (fp16)
            wo_sb = sp.tile([128, DC, 2 * D], f32, tag="wo_sb")
            nc.sync.dma_start(out=wo_sb[:],
                              in_=wo_d.rearrange("(c p) k -> p c k", p=128))
            wo_h = sp.tile([128, DC, 2 * D], f16, tag="wo_h")
            nc.vector.tensor_copy(wo_h[:].rearrange("p c k -> p (c k)"),
                                  wo_sb[:].rearrange("p c k -> p (c k)"))
            for i in range(KO):
                pwo = ps_pre.tile([128, D], f16, tag="pre_h")
                for c in range(DC):
                    nc.tensor.transpose(pwo[:, c * 128:(c + 1) * 128],
                                        wo_h[:, c, i * 128:(i + 1) * 128], ident_h[:])
                nc.vector.tensor_copy(woT[:, i, :], pwo[:])

            # WvT (fp16)
            pwv = ps_pre.tile([128, DC], f32, tag="pre_s")
            for c in range(DC):
                nc.tensor.transpose(pwv[:, c:c + 1], wv_sb[0:1, c * 128:(c + 1) * 128],
                                    ident_f[0:1, 0:1])
            nc.vector.tensor_copy(wvT[:], pwv[:])

            # qq = output @ Wq.T + bq + bk  (fp32, one-off), transposed columns
            wq_sb = sp.tile([128, DC, D], f32, tag="wq_sb")
            nc.sync.dma_start(out=wq_sb[:],
                              in_=wq_d.rearrange("(c p) d -> p c d", p=128))
            wqT = sp.tile([128, DC, D], f32, tag="wqT")
            for i in range(DC):
                pwq = ps_pre.tile([128, D], f32, tag="pre_f")
                for c in range(DC):
                    nc.tensor.transpose(pwq[:, c * 128:(c + 1) * 128],
                                        wq_sb[:, c, i * 128:(i + 1) * 128], ident_f[:])
                nc.vector.tensor_copy(wqT[:, i, :], pwq[:])
            outT = sp.tile([128, DC, BPC], f32, tag="outT")
            pot = ps_pre.tile([128, DC * BPC], f32, tag="pre_s")
            for i in range(DC):
                nc.tensor.transpose(pot[:, i * BPC:(i + 1) * BPC],
                                    outp_sb[0:BPC, i * 128:(i + 1) * 128],
                                    ident_f[0:BPC, 0:BPC])
            nc.vector.tensor_copy(outT[:].rearrange("p c b -> p (c b)"), pot[:])
            ones14 = sp.tile([1, BPC], f32, tag="ones14")
            nc.gpsimd.memset(ones14[:], 1.0)
            pqq = ps_pre.tile([BPC, D], f32, tag="pre_q")
            for i in range(DC):
                nc.tensor.matmul(pqq[:], outT[:, i, :], wqT[:, i, :],
                                 start=(i == 0), stop=False)
            nc.tensor.matmul(pqq[:], ones14[:], bqk[:], start=False, stop=True)
            qq_sb = sp.tile([BPC, D], f32, tag="qq_sb")
            nc.vector.tensor_copy(qq_sb[:], pqq[:])
            pqt = ps_pre.tile([128, DC * BPC], f32, tag="pre_s")
            for c in range(DC):
                nc.tensor.transpose(pqt[:, c * BPC:(c + 1) * BPC],
                                    qq_sb[0:BPC, c * 128:(c + 1) * 128],
                                    ident_f[0:BPC, 0:BPC])
            nc.vector.tensor_copy(qqT[:].rearrange("p c b -> p (c b)"), pqt[:])

        # ---- main loop (software-pipelined: group chain deferred one step) ----
        with tc.tile_pool(name="ctxT", bufs=2) as ctxT_pool, \
             tc.tile_pool(name="feats", bufs=2) as feats_pool, \
             tc.tile_pool(name="sb_small", bufs=2) as sbs, \
             tc.tile_pool(name="ps_k", bufs=2, space="PSUM") as ps_k, \
             tc.tile_pool(name="ps_tr", bufs=2, space="PSUM") as ps_tr, \
             tc.tile_pool(name="ps_m", bufs=1, space="PSUM") as ps_m, \
             tc.tile_pool(name="ps_s", bufs=1, space="PSUM") as ps_s:

            bstate = {}

            def start_batch(b):
                psum_m = ps_m.tile([128, D], f32, tag="mix", name="psum_m")
                nc.scalar.memzero(psum_m[:])
                bstate[b] = {
                    "psum_m": psum_m,
                    "attn_cols": sbs.tile([128, NG * NTC], f32,
                                          tag="attn_cols", name="attn_cols"),
                    "esums": sbs.tile([128, NG], f32, tag="esums", name="esums"),
                }

            def emit_tr_quad(ctx_g, ctxT_g, q, dc):
                ptr = ps_tr.tile([128, 512], f16, tag="tr", name="ptr")
                for j in range(4):
                    n = q * 4 + j
                    nc.tensor.transpose(
                        ptr[:, j * 128:(j + 1) * 128],
                        ctx_g[:, n, dc * 128:(dc + 1) * 128],
                        ident_h[:])
                nc.vector.tensor_copy(
                    ctxT_g[:, dc, q * 512:(q + 1) * 512], ptr[:])

            def emit_keys_unit(b, ctxT_g, feats_g, mc, h):
                pk = ps_k.tile([128, TG // 2], f32, tag="k", name="pk")
                for kc in range(DC):
                    for th in range(2):
                        t0 = th * 512
                        nc.tensor.matmul(
                            pk[:, t0:t0 + 512],
                            wkT[:, kc, mc * 128:(mc + 1) * 128],
                            ctxT_g[:, kc, h * 1024 + t0:h * 1024 + t0 + 512],
                            start=(kc == 0), stop=(kc == DC - 1),
                            skip_group_check=True)
                nc.scalar.activation(
                    feats_g[:, mc, h * 1024:(h + 1) * 1024], pk[:],
                    AF.Tanh, bias=qqT[:, mc, b:b + 1])

            def chain_scores(b, g, feats_g, pool=None):
                st = bstate[b]
                psc = (pool or ps_s).tile([128, 512], f32, tag="sc" if pool is None else "tr", name="psc")
                nc.scalar.memzero(psc[:])
                for mc in range(DC):
                    for j in range(NTS):
                        nc.tensor.matmul(
                            psc[32 * j:32 * j + 1, :],
                            wvT[:, mc:mc + 1],
                            feats_g[:, mc, j * 512:(j + 1) * 512],
                            start=(mc == 0), stop=(mc == DC - 1),
                            tile_position=(0, 32 * j),
                            skip_group_check=True)
                e_sb = sbs.tile([128, 512], f32, tag="e_sb", name="e_sb")
                nc.scalar.activation(
                    e_sb[:], psc[:], AF.Exp, bias=bvs_bc[:, 0:1],
                    accum_out=st["esums"][:, g:g + 1])
                return e_sb

            def chain_etr(b, g, e_sb, pool=None):
                st = bstate[b]
                pe = (pool or ps_s).tile([128, NTC], f32, tag="sc" if pool is None else "tr", name="pe")
                for j in range(NTS):
                    for k in range(4):
                        n = j * 4 + k
                        nc.tensor.transpose(
                            pe[:, n:n + 1],
                            e_sb[32 * j:32 * j + 1, k * 128:(k + 1) * 128],
                            ident_f[32 * j:32 * j + 1, 32 * j:32 * j + 1],
                            tile_position=(32 * j, 0))
                ecol = sbs.tile([128, NTC], f16, tag="ecol", name="ecol")
                nc.vector.tensor_copy(ecol[:], pe[:])
                nc.vector.tensor_copy(st["attn_cols"][:, g * NTC:(g + 1) * NTC], pe[:])
                return ecol

            def chain_mix(b, g, ctx_g, ecol):
                st = bstate[b]
                for n in range(NTC):
                    j = n % 4
                    nc.tensor.matmul(
                        st["psum_m"][32 * j:32 * j + 1, :],
                        ecol[:, n:n + 1],
                        ctx_g[:, n, :],
                        start=False, stop=(g == NG - 1 and n >= NTC - 4),
                        tile_position=(0, 32 * j),
                        skip_group_check=True)

            def chain(b, g, ctx_g, feats_g):
                e_sb = chain_scores(b, g, feats_g)
                ecol = chain_etr(b, g, e_sb)
                chain_mix(b, g, ctx_g, ecol)

            def tail(b):
                st = bstate.pop(b)
                psum_m, attn_cols, esums = (st["psum_m"], st["attn_cols"],
                                            st["esums"])
                # total sum of e: masked partition-sum of per-partition sums
                esums_h = sbs.tile([128, NG], f16, tag="esums_h")
                nc.vector.tensor_copy(esums_h[:], esums[:])
                pes = ps_s.tile([1, NG], f32, tag="sc", name="pes")
                nc.tensor.matmul(pes[:], maskv[:], esums_h[:],
                                 start=True, stop=True)
                erow = sbs.tile([1, NG], f32, tag="erow")
                nc.vector.tensor_copy(erow[:], pes[:])
                esumT = sbs.tile([1, 1], f32, tag="esumT")
                nc.vector.reduce_sum(esumT[:], erow[:], axis=mybir.AxisListType.X)
                recip = sbs.tile([1, 1], f32, tag="recip")
                nc.vector.reciprocal(recip[:], esumT[:])

                # broadcast recip to [128,1] via tiny fp32 matmul
                pbr = ps_s.tile([128, 1], f32, tag="sc", name="pbr")
                nc.tensor.matmul(pbr[:], ones_f[0:1, :], recip[0:1, 0:1],
                                 start=True, stop=True)
                recip_bc = sbs.tile([128, 1], f32, tag="recip_bc")
                nc.vector.tensor_copy(recip_bc[:], pbr[:])

                # attn = e * recip, transposed back to rows, DMA out
                attn_n = sbs.tile([128, NG * NTC], f32, tag="attn_n")
                nc.scalar.activation(attn_n[:], attn_cols[:], AF.Copy,
                                     scale=recip_bc[:, 0:1])
                pa = ps_s.tile([NG * NTC, 128], f32, tag="sc", name="pa")
                nc.tensor.transpose(pa[:], attn_n[:], ident_f[:])
                attn_out = sbs.tile([NG * NTC, 128], f32, tag="attn_out")
                nc.vector.tensor_copy(attn_out[:], pa[:])
                nc.sync.dma_start(
                    out=o_attn[b].rearrange("o (a c) -> (o a) c", c=128),
                    in_=attn_out[:])

                # mix: sum 4 column partials (garbage rows are zero)
                pmh = sbs.tile([128, D], f16, tag="pmh")
                nc.vector.tensor_copy(pmh[:], psum_m[:])
                pms = ps_s.tile([1, D], f32, tag="sc", name="pms")
                nc.tensor.matmul(pms[:], ones_col_h[:], pmh[:],
                                 start=True, stop=True)
                mix_sb = sbs.tile([1, D], f32, tag="mix_sb")
                nc.scalar.activation(mix_sb[:], pms[0:1, :], AF.Copy,
                                     scale=recip[0:1, 0:1])
                nc.sync.dma_start(out=o_mix[b].rearrange("o d -> (o d)"),
                                  in_=mix_sb[0:1, :])

                # out = tanh([mix, output] @ Wo.T + bo)
                pc = ps_s.tile([128, KO], f32, tag="sc", name="pc")
                for i in range(DC):
                    nc.tensor.transpose(pc[:, i:i + 1],
                                        mix_sb[0:1, i * 128:(i + 1) * 128],
                                        ident_f[0:1, 0:1])
                for i in range(DC):
                    nc.tensor.transpose(
                        pc[:, DC + i:DC + i + 1],
                        outp_row[0:1, b * D + i * 128:b * D + (i + 1) * 128],
                        ident_f[0:1, 0:1])
                comb = sbs.tile([128, KO], f16, tag="comb")
                nc.vector.tensor_copy(comb[:], pc[:])
                po = ps_s.tile([1, D], f32, tag="sc", name="po")
                for i in range(KO):
                    nc.tensor.matmul(po[:], comb[:, i:i + 1], woT[:, i, :],
                                     start=(i == 0), stop=False,
                                     skip_group_check=True)
                nc.tensor.matmul(po[:], one1_h[:], bo_h[:], start=False, stop=True,
                                 skip_group_check=True)
                out_sb = sbs.tile([1, D], f32, tag="out_sb")
                nc.scalar.activation(out_sb[:], po[0:1, :], AF.Tanh)
                nc.sync.dma_start(out=o_out[b].rearrange("o d -> (o d)"),
                                  in_=out_sb[0:1, :])

            NQ = NTC // 4  # tr quad columns per g
            state = {}     # i -> dict(b, g, ctx, ctxT, feats)
            n_groups = len(groups)

            def emit_iteration(i):
                # stage A for group i: transposes; stage B for group i-1: keys;
                # stage C for group i-2: scores/softmax/mix chain (+ batch tail)
                if i < n_groups:
                    b, g = groups[i]
                    if g == 0:
                        start_batch(b)
                    ctx_g = prefetched.pop((b, g))
                    ctxT_g = ctxT_pool.tile([128, DC, TG], f16, tag="ctxT",
                                            name="ctxT_g")
                    state[i] = {"b": b, "g": g, "ctx": ctx_g, "ctxT": ctxT_g}
                    tr_units = [(q, dc) for q in range(NQ) for dc in range(DC)]
                else:
                    tr_units = []
                if 0 <= i - 1 < n_groups:
                    st1 = state[i - 1]
                    st1["feats"] = feats_pool.tile([128, DC, TG], f16,
                                                   tag="feats", name="feats_g")
                    k_units = [(mc, h) for mc in range(DC) for h in range(2)]
                else:
                    st1, k_units = None, []
                # interleave: 2 tr quads per keys unit
                nk = len(k_units)
                for s in range(max(nk, (len(tr_units) + 1) // 2)):
                    for t in tr_units[2 * s:2 * s + 2]:
                        emit_tr_quad(state[i]["ctx"], state[i]["ctxT"], *t)
                    if s < nk:
                        mc, h = k_units[s]
                        emit_keys_unit(st1["b"], st1["ctxT"], st1["feats"], mc, h)
                if i + 1 < n_groups and groups[i + 1] not in prefetched:
                    prefetched[groups[i + 1]] = load_ctx(*groups[i + 1])
                if 0 <= i - 2 < n_groups:
                    st2 = state.pop(i - 2)
                    chain(st2["b"], st2["g"], st2["ctx"], st2["feats"])
                    if st2["g"] == NG - 1:
                        tail(st2["b"])

            for i in range(n_groups):
                emit_iteration(i)
            # drain: keys for the last group, then the final two chains
            # interleaved so their ACT/DVE round-trips hide behind PE work
            stA = state.pop(n_groups - 2)
            stB = state.pop(n_groups - 1)
            stB["feats"] = feats_pool.tile([128, DC, TG], f16,
                                           tag="feats", name="feats_g")
            for mc in range(DC):
                for h in range(2):
                    emit_keys_unit(stB["b"], stB["ctxT"], stB["feats"], mc, h)
            eA = chain_scores(stA["b"], stA["g"], stA["feats"], pool=ps_tr)
            eB = chain_scores(stB["b"], stB["g"], stB["feats"], pool=ps_tr)
            ecA = chain_etr(stA["b"], stA["g"], eA, pool=ps_tr)
            ecB = chain_etr(stB["b"], stB["g"], eB, pool=ps_tr)
            chain_mix(stA["b"], stA["g"], stA["ctx"], ecA)
            chain_mix(stB["b"], stB["g"], stB["ctx"], ecB)
            if stA["g"] == NG - 1:
                tail(stA["b"])
            tail(stB["b"])
        ctx_pool_cm.__exit__(None, None, None)


def _get_nc():
    if "nc" not in _CACHE:
        _CACHE["nc"] = _build()
    return _CACHE["nc"]


def _run_spmd(in_maps, **kw):
    nc = _get_nc()
    return run_bass_kernel_spmd(nc, in_maps, list(range(N_CORES)), **kw)


def _make_in_maps(output, context, Wq, bq, Wk, bk, Wv, bv, Wo, bo):
    arrs = {
        "Wq": np.ascontiguousarray(np.asarray(Wq, np.float32)),
        "bq": np.ascontiguousarray(np.asarray(bq, np.float32)),
        "Wk": np.ascontiguousarray(np.asarray(Wk, np.float32)),
        "bk": np.ascontiguousarray(np.asarray(bk, np.float32)),
        "Wv": np.ascontiguousarray(np.asarray(Wv, np.float32)),
        "bv": np.ascontiguousarray(np.asarray(bv, np.float32)),
        "Wo": np.ascontiguousarray(np.asarray(Wo, np.float32)),
        "bo": np.ascontiguousarray(np.asarray(bo, np.float32)),
    }
    output = np.ascontiguousarray(np.asarray(output, np.float32))
    context = np.ascontiguousarray(np.asarray(context, np.float32))
    in_maps = []
    for c in range(N_CORES):
        sl = slice(c * BPC, (c + 1) * BPC)
        m = dict(arrs)
        m["output"] = np.ascontiguousarray(output[sl])
        m["context"] = np.ascontiguousarray(context[sl])
        in_maps.append(m)
    return in_maps


def kernel(output, context, Wq, bq, Wk, bk, Wv, bv, Wo, bo):
    in_maps = _make_in_maps(output, context, Wq, bq, Wk, bk, Wv, bv, Wo, bo)
    res = _run_spmd(in_maps)
    out = np.concatenate([r["o_out"] for r in res.results], axis=0)
    attn = np.concatenate([r["o_attn"] for r in res.results], axis=0)
    mix = np.concatenate([r["o_mix"] for r in res.results], axis=0)
    return (out, attn, mix)
